# revision 35
# baseline (speedup 1.0000x reference)
"""Trainium2 Bass kernel for nn_CrossAttention_78305843740743.

Computes, for query [B, Q, Dq] and key [B, K, Dk]:
    ql = query @ W_lq + b_lq                  # [B, Q, D]
    kl = key   @ W_lk + b_lk                  # [B, K, D]
    lin[b,q,k]  = sum_d v_d * tanh(ql[b,q,d] + kl[b,k,d]) + b_att
    qb = query @ W_bq + b_bq
    kb = key   @ W_bk + b_bk
    bi[b,q,k]   = (qb . kb) / sqrt(D)
    out = lin + bi                            # [B, Q, K]

Sharding: 8 cores = (batch b in {0,1}) x (4 query chunks of 64). Each core
gets its query slab + the full key[b] + all (small) weights, and produces a
[64, 2048] slab of the output. No collectives.

Per-core dataflow (d=128 lives on SBUF partitions):
  - key is DMA'd first (4 batched 1MB transfers on 2 queues), cast to fp16,
    transposed on PE (fp16 = 1 cycle/row), and projected to klT [128, 2048]
    (fp32, biases folded) -- the only dependency of the ACT pipeline,
  - linear path: per q one ACT instr  tanh(klT + bias=qlT[:, q])  [128, 2048]
    writing fp16, then 4 matmuls with a shifted-diagonal stationary (v_att in
    column q of a [128, 64] fp16 window) accumulate v.tanh(...) into PSUM row
    q on top of the bilinear result,
  - kbT projection + the bilinear matmul (start=True into the 4 output PSUM
    banks) run in the ACT pipeline's shadow,
  - evacuate PSUM + b_att on DVE, DMA out.

ACT (tanh over 16.8M elements/core at 1 elem/lane/cycle @ 1.2 GHz,
dtype-independent) is the roofline for this op: ~122us/core busy. All
weights/biases are packed host-side into 2 tensors to minimize the ~0.65us
per-DMA descriptor-generation cost on the issuing sequencers.
"""

import math
from contextlib import ExitStack

import numpy as np

import concourse.bacc as bacc
import concourse.bass as bass
import concourse.tile as tile
from concourse import mybir
from concourse.bass_utils import run_bass_kernel_spmd
from concourse.masks import make_identity

F32 = mybir.dt.float32
F16 = mybir.dt.float16
P = 128

BSZ, NUM_Q, NUM_K = 2, 256, 2048
D_Q, D_K, D_ATT = 512, 512, 128
N_CORES = 8
Q_CHUNKS = 4
Q_SHARD = NUM_Q // Q_CHUNKS  # 64 queries per core
KO = D_Q // P                # 4 contraction chunks for the input projections
KT = NUM_K // P              # 16 key tiles of 128
KB = 4                       # key DMA batches (KT/KB tiles each)
NB = NUM_K // 512            # 4 psum banks of [64, 512] for the output slab

_CACHED = {}


def _build_bass(n_iters: int = 1) -> bass.Bass:
    nc = bacc.Bacc("TRN2", target_bir_lowering=False, debug=False,
                   num_devices=N_CORES)

    # weights host-prepacked into the exact SBUF layouts (partition-major)
    # so the DMAs are plain contiguous copies: W_lk alone (needed earliest)
    wlk_d = nc.dram_tensor("wlk", [P, KO, D_ATT], F32, kind="ExternalInput").ap()
    # packed [128, 3, 4, 128] = (W_lq, W_bq, W_bk) partition-major
    w3_d = nc.dram_tensor("w3", [P, 3, KO, D_ATT], F32, kind="ExternalInput").ap()
    # packed vectors: [128, 6] = (b_lq+b_lk, b_bq, b_bk, v_att, b_att, 0)
    vec_d = nc.dram_tensor("vec", [P, 6], F32, kind="ExternalInput").ap()
    query_d = nc.dram_tensor("query_s", [Q_SHARD, D_Q], F32, kind="ExternalInput").ap()
    key_d = nc.dram_tensor("key_b", [NUM_K, D_K], F32, kind="ExternalInput").ap()
    out_d = nc.dram_tensor("out", [Q_SHARD, NUM_K], F32, kind="ExternalOutput").ap()

    with tile.TileContext(nc) as tc, ExitStack() as ctx:
        if n_iters > 1:
            # benchmarking only: repeat the whole kernel body in-NEFF so
            # per-iteration time can be measured as a wall-clock delta
            ctx.enter_context(tc.For_i(0, n_iters, 1,
                                       hint_engines=(mybir.EngineType.PE,)))
        singles = ctx.enter_context(tc.tile_pool(name="singles", bufs=1))
        knat_pool = ctx.enter_context(tc.tile_pool(name="knat", bufs=6))
        keyt_pool = ctx.enter_context(tc.tile_pool(name="keyt", bufs=KT))
        tmp_pool = ctx.enter_context(tc.tile_pool(name="tmp", bufs=8))
        # two tags in this pool: "wk" [128, 512] transpose batches and "pj"
        # [128, 128] projection accumulators -> 2 banks each, + 4 output
        # banks = 8 PSUM banks exactly
        wk_psum = ctx.enter_context(tc.tile_pool(name="wk_psum", bufs=2, space="PSUM"))
        out_psum = ctx.enter_context(tc.tile_pool(name="out_psum", bufs=NB, space="PSUM"))

        # ---- DMA plan: key tiles first (they gate everything), simple
        # contiguous per-tile transfers spread over 3 queues ----
        knats = []
        dma_engs = [nc.sync, nc.gpsimd, nc.scalar]
        for kt in range(4):
            knat = knat_pool.tile([P, D_K], F32, tag="knat", name=f"knat_{kt}")
            dma_engs[kt % 3].dma_start(out=knat,
                                       in_=key_d[kt * P:(kt + 1) * P, :])
            knats.append(knat)

        wlk32 = singles.tile([P, KO, D_ATT], F32, tag="wlk32")
        nc.sync.dma_start(out=wlk32, in_=wlk_d)
        wlk16 = singles.tile([P, KO, D_ATT], F16, tag="wlk16")
        nc.vector.tensor_copy(out=wlk16, in_=wlk32)

        vec = singles.tile([P, 6], F32, tag="vec")
        nc.scalar.dma_start(out=vec, in_=vec_d)
        bsum = vec[:, 0:1]   # b_lq + b_lk (packed on host)
        bbq = vec[:, 1:2]
        bbk = vec[:, 2:3]
        vT = vec[:, 3:4]
        batt = vec[:, 4:5]   # b_att broadcast along partitions (host-packed)

        qnat = singles.tile([P, D_Q], F32, tag="qnat")
        nc.vector.memset(qnat, 0.0)
        nc.scalar.dma_start(out=qnat[:Q_SHARD, :], in_=query_d)

        for kt in range(4, KT):
            knat = knat_pool.tile([P, D_K], F32, tag="knat", name=f"knat_{kt}")
            dma_engs[kt % 3].dma_start(out=knat,
                                       in_=key_d[kt * P:(kt + 1) * P, :])
            knats.append(knat)

        # remaining weights (needed only after klT): one packed DMA
        w32 = singles.tile([P, 3, KO, D_ATT], F32, tag="w32")
        nc.gpsimd.dma_start(out=w32, in_=w3_d)
        w16 = singles.tile([P, 3, KO, D_ATT], F16, tag="w16")
        nc.vector.tensor_copy(out=w16, in_=w32)
        WLQ, WBQ, WBK = 0, 1, 2

        identity = singles.tile([P, P], F32)
        make_identity(nc, identity)
        id16 = singles.tile([P, P], F16, tag="id16")
        nc.vector.tensor_copy(out=id16, in_=identity)

        # v_att embedded at column Q_SHARD-1 of a [P, 2*Q_SHARD-1] buffer:
        # the [:, Q_SHARD-1-q : 2*Q_SHARD-1-q] window has v at local column q.
        # fp16 so the linear-path matmuls stream at 1 cycle/row (fp32 is 4x).
        vdiag = singles.tile([P, 2 * Q_SHARD - 1], F16, tag="vdiag")
        nc.vector.memset(vdiag, 0.0)
        nc.vector.tensor_copy(out=vdiag[:, Q_SHARD - 1:Q_SHARD], in_=vT)

        # ---- query transposes + projections (fp32: tiny and off the
        # critical path, so keep full precision on the q side) ----
        qT = singles.tile([P, KO, Q_SHARD], F32, tag="qT")
        pbq = wk_psum.tile([P, KO * P], F32, tag="wk", name="ptq")
        for c in range(KO):
            nc.tensor.transpose(pbq[:, c * P:(c + 1) * P],
                                qnat[:, c * P:(c + 1) * P], identity)
        nc.vector.tensor_copy(
            out=qT, in_=pbq.rearrange("p (c k) -> p c k", c=KO)[:, :, :Q_SHARD])

        qlT = singles.tile([P, Q_SHARD], F32, tag="qlT")
        pql = wk_psum.tile([P, P], F32, tag="pj")
        for c in range(KO):
            nc.tensor.matmul(pql[:, :Q_SHARD], w32[:, WLQ, c, :], qT[:, c, :],
                             start=(c == 0), stop=(c == KO - 1))
        nc.vector.tensor_copy(out=qlT, in_=pql[:, :Q_SHARD])

        qbT = singles.tile([P, Q_SHARD], F16, tag="qbT")
        pqb = wk_psum.tile([P, P], F32, tag="pj")
        for c in range(KO):
            nc.tensor.matmul(pqb[:, :Q_SHARD], w32[:, WBQ, c, :], qT[:, c, :],
                             start=(c == 0), stop=(c == KO - 1))
        # qb scaled by 1/sqrt(D_ATT) (bilinear normalizer), bias first
        nc.vector.tensor_scalar(out=qbT, in0=pqb[:, :Q_SHARD], scalar1=bbq,
                                scalar2=1.0 / math.sqrt(D_ATT),
                                op0=mybir.AluOpType.add,
                                op1=mybir.AluOpType.mult)

        # ---- key pipeline per tile: 4 fp32 PE transposes into one PSUM
        # bank, a single fp16-casting evac (alternating DVE/ACT -- ACT only
        # absorbs what fits in its otherwise-idle prologue window), fp16
        # klT projection, DVE bias-fold ----
        klT = singles.tile([P, NUM_K], F32, tag="klT")
        kbT = singles.tile([P, NUM_K], F16, tag="kbT")
        keyts = []
        for kt in range(KT):
            pb = wk_psum.tile([P, KO * P], F32, tag="wk", name=f"ptk_{kt}")
            for c in range(KO):
                nc.tensor.transpose(pb[:, c * P:(c + 1) * P],
                                    knats[kt][:, c * P:(c + 1) * P], identity)
            keyt = keyt_pool.tile([P, KO, P], F16, tag="keyt",
                                  name=f"keyt_{kt}")
            pbv = pb.rearrange("p (c k) -> p c k", c=KO)
            if kt % 2 == 0:
                nc.vector.tensor_copy(out=keyt, in_=pbv)
            else:
                nc.scalar.copy(out=keyt, in_=pbv)
            keyts.append(keyt)
            pkl = wk_psum.tile([P, P], F32, tag="pj", name=f"pkl_{kt}")
            for c in range(KO):
                nc.tensor.matmul(pkl, wlk16[:, c, :], keyt[:, c, :],
                                 start=(c == 0), stop=(c == KO - 1))
            nc.vector.tensor_scalar_add(out=klT[:, kt * P:(kt + 1) * P],
                                        in0=pkl, scalar1=bsum)

        # ---- kbT projection (runs in the ACT pipeline's shadow) ----
        for kt in range(KT):
            pkb = wk_psum.tile([P, P], F32, tag="wk", name=f"pkb_{kt}")
            for c in range(KO):
                nc.tensor.matmul(pkb, w16[:, WBK, c, :], keyts[kt][:, c, :],
                                 start=(c == 0), stop=(c == KO - 1))
            nc.vector.tensor_scalar_add(out=kbT[:, kt * P:(kt + 1) * P],
                                        in0=pkb, scalar1=bbk)

        # ---- bilinear baseline into the 4 output psum banks ----
        po = [out_psum.tile([Q_SHARD, 512], F32, tag="po", name=f"po_{i}")
              for i in range(NB)]
        for i in range(NB):
            nc.tensor.matmul(po[i], qbT, kbT[:, i * 512:(i + 1) * 512],
                             start=True, stop=False)

        # ---- linear (tanh) path, accumulated on top ----
        for q in range(Q_SHARD):
            tmp = tmp_pool.tile([P, NUM_K], F16, tag="tmp")
            nc.scalar.activation(tmp, klT, mybir.ActivationFunctionType.Tanh,
                                 bias=qlT[:, q:q + 1], scale=1.0)
            last = q == Q_SHARD - 1
            sta = vdiag[:, Q_SHARD - 1 - q:2 * Q_SHARD - 1 - q]
            for i in range(NB):
                nc.tensor.matmul(po[i], sta, tmp[:, i * 512:(i + 1) * 512],
                                 start=False, stop=last)

        # ---- + b_att, evacuate, store ----
        out_sb = singles.tile([Q_SHARD, NUM_K], F32, tag="out_sb")
        for i in range(NB):
            nc.vector.tensor_scalar_add(out=out_sb[:, i * 512:(i + 1) * 512],
                                        in0=po[i], scalar1=batt[:Q_SHARD])
        nc.sync.dma_start(out=out_d, in_=out_sb)

    nc.compile()
    return nc


def _get_nc() -> bass.Bass:
    if "nc" not in _CACHED:
        _CACHED["nc"] = _build_bass()
    return _CACHED["nc"]


def make_in_maps(**inputs) -> list[dict[str, np.ndarray]]:
    f = lambda x: np.ascontiguousarray(np.asarray(x, dtype=np.float32))
    query = f(inputs["query"])
    key = f(inputs["key"])
    # pre-pack weights partition-major: [ko*128+p, d] -> [p, ko, d]
    pack = lambda w: np.ascontiguousarray(
        f(w).reshape(KO, P, D_ATT).transpose(1, 0, 2))
    wlk = pack(inputs["W_lk"])
    w3 = np.ascontiguousarray(np.stack(
        [pack(inputs["W_lq"]), pack(inputs["W_bq"]), pack(inputs["W_bk"])],
        axis=1))  # [128, 3, 4, 128]
    vec = np.zeros((6, D_ATT), np.float32)
    vec[0] = f(inputs["b_lq"]) + f(inputs["b_lk"])
    vec[1] = f(inputs["b_bq"])
    vec[2] = f(inputs["b_bk"])
    vec[3] = f(inputs["v_att"])
    vec[4] = np.float32(np.asarray(inputs["b_att"], np.float32).reshape(()))
    vec = np.ascontiguousarray(vec.T)  # [128, 6]
    shared = {"wlk": wlk, "w3": w3, "vec": vec}
    in_maps = []
    for c in range(N_CORES):
        b, qc = divmod(c, Q_CHUNKS)
        in_maps.append({
            "query_s": np.ascontiguousarray(query[b, qc * Q_SHARD:(qc + 1) * Q_SHARD, :]),
            "key_b": np.ascontiguousarray(key[b]),
            **shared,
        })
    return in_maps


def assemble(results: list[dict[str, np.ndarray]]) -> np.ndarray:
    out = np.empty((BSZ, NUM_Q, NUM_K), np.float32)
    for c in range(N_CORES):
        b, qc = divmod(c, Q_CHUNKS)
        out[b, qc * Q_SHARD:(qc + 1) * Q_SHARD, :] = results[c]["out"]
    return out


def kernel(**inputs) -> np.ndarray:
    nc = _get_nc()
    in_maps = make_in_maps(**inputs)
    res = run_bass_kernel_spmd(nc, in_maps, list(range(N_CORES)))
    return assemble(res.results)


# revision 40
# speedup vs baseline: 1.1855x; 1.1855x over previous
"""Trainium2 Bass kernel for nn_CrossAttention_78305843740743.

Computes, for query [B, Q, Dq] and key [B, K, Dk]:
    ql = query @ W_lq + b_lq                  # [B, Q, D]
    kl = key   @ W_lk + b_lk                  # [B, K, D]
    lin[b,q,k]  = sum_d v_d * tanh(ql[b,q,d] + kl[b,k,d]) + b_att
    qb = query @ W_bq + b_bq
    kb = key   @ W_bk + b_bk
    bi[b,q,k]   = (qb . kb) / sqrt(D)
    out = lin + bi                            # [B, Q, K]

Sharding: 8 cores = (batch b in {0,1}) x (4 query chunks of 64). Each core
gets its query slab + the full key[b] + all (small) weights, and produces a
[64, 2048] slab of the output. No collectives.

Per-core dataflow (d=128 lives on SBUF partitions):
  - key is DMA'd first (4 batched 1MB transfers on 2 queues), cast to fp16,
    transposed on PE (fp16 = 1 cycle/row), and projected to klT [128, 2048]
    (fp32, biases folded) -- the only dependency of the ACT pipeline,
  - linear path: per q one ACT instr  tanh(klT + bias=qlT[:, q])  [128, 2048]
    writing fp16, then 4 matmuls with a per-q aligned stationary slab (v_att in
    column q of vsta[:, q, :], zeros elsewhere) accumulate v.tanh() into PSUM row
    q on top of the bilinear result,
  - kbT projection + the bilinear matmul (start=True into the 4 output PSUM
    banks) run in the ACT pipeline's shadow,
  - evacuate PSUM + b_att on DVE, DMA out.

ACT (tanh over 16.8M elements/core at 1 elem/lane/cycle @ 1.2 GHz,
dtype-independent) is the roofline for this op: ~122us/core busy. All
weights/biases are packed host-side into 2 tensors to minimize the ~0.65us
per-DMA descriptor-generation cost on the issuing sequencers.
"""

import math
from contextlib import ExitStack

import numpy as np

import concourse.bacc as bacc
import concourse.bass as bass
import concourse.tile as tile
from concourse import mybir
from concourse.bass_utils import run_bass_kernel_spmd
from concourse.masks import make_identity

F32 = mybir.dt.float32
F16 = mybir.dt.float16
P = 128

BSZ, NUM_Q, NUM_K = 2, 256, 2048
D_Q, D_K, D_ATT = 512, 512, 128
N_CORES = 8
Q_CHUNKS = 4
Q_SHARD = NUM_Q // Q_CHUNKS  # 64 queries per core
KO = D_Q // P                # 4 contraction chunks for the input projections
KT = NUM_K // P              # 16 key tiles of 128
KB = 4                       # key DMA batches (KT/KB tiles each)
NB = NUM_K // 512            # 4 psum banks of [64, 512] for the output slab

_CACHED = {}


def _build_bass(n_iters: int = 1) -> bass.Bass:
    nc = bacc.Bacc("TRN2", target_bir_lowering=False, debug=False,
                   num_devices=N_CORES)

    # weights host-prepacked into the exact SBUF layouts (partition-major)
    # so the DMAs are plain contiguous copies: W_lk alone (needed earliest)
    wlk_d = nc.dram_tensor("wlk", [P, KO, D_ATT], F32, kind="ExternalInput").ap()
    # packed [128, 3, 4, 128] = (W_lq, W_bq, W_bk) partition-major
    w3_d = nc.dram_tensor("w3", [P, 3, KO, D_ATT], F32, kind="ExternalInput").ap()
    # packed vectors: [128, 6] = (b_lq+b_lk, b_bq, b_bk, v_att, b_att, 0)
    vec_d = nc.dram_tensor("vec", [P, 6], F32, kind="ExternalInput").ap()
    query_d = nc.dram_tensor("query_s", [Q_SHARD, D_Q], F32, kind="ExternalInput").ap()
    key_d = nc.dram_tensor("key_b", [NUM_K, D_K], F32, kind="ExternalInput").ap()
    out_d = nc.dram_tensor("out", [Q_SHARD, NUM_K], F32, kind="ExternalOutput").ap()

    with tile.TileContext(nc) as tc, ExitStack() as ctx:
        if n_iters > 1:
            # benchmarking only: repeat the whole kernel body in-NEFF so
            # per-iteration time can be measured as a wall-clock delta
            ctx.enter_context(tc.For_i(0, n_iters, 1,
                                       hint_engines=(mybir.EngineType.PE,)))
        singles = ctx.enter_context(tc.tile_pool(name="singles", bufs=1))
        knat_pool = ctx.enter_context(tc.tile_pool(name="knat", bufs=6))
        keyt_pool = ctx.enter_context(tc.tile_pool(name="keyt", bufs=KT))
        tmp_pool = ctx.enter_context(tc.tile_pool(name="tmp", bufs=8))
        # two tags in this pool: "wk" [128, 512] transpose batches and "pj"
        # [128, 128] projection accumulators -> 2 banks each, + 4 output
        # banks = 8 PSUM banks exactly
        wk_psum = ctx.enter_context(tc.tile_pool(name="wk_psum", bufs=2, space="PSUM"))
        out_psum = ctx.enter_context(tc.tile_pool(name="out_psum", bufs=NB, space="PSUM"))

        # ---- DMA plan: key tiles first (they gate everything), simple
        # contiguous per-tile transfers spread over 3 queues ----
        knats = []
        dma_engs = [nc.sync, nc.gpsimd, nc.scalar]
        for kt in range(4):
            knat = knat_pool.tile([P, D_K], F32, tag="knat", name=f"knat_{kt}")
            dma_engs[kt % 3].dma_start(out=knat,
                                       in_=key_d[kt * P:(kt + 1) * P, :])
            knats.append(knat)

        wlk32 = singles.tile([P, KO, D_ATT], F32, tag="wlk32")
        nc.sync.dma_start(out=wlk32, in_=wlk_d)
        wlk16 = singles.tile([P, KO, D_ATT], F16, tag="wlk16")
        nc.vector.tensor_copy(out=wlk16, in_=wlk32)

        vec = singles.tile([P, 6], F32, tag="vec")
        nc.scalar.dma_start(out=vec, in_=vec_d)
        bsum = vec[:, 0:1]   # b_lq + b_lk (packed on host)
        bbq = vec[:, 1:2]
        bbk = vec[:, 2:3]
        vT = vec[:, 3:4]
        batt = vec[:, 4:5]   # b_att broadcast along partitions (host-packed)

        qnat = singles.tile([P, D_Q], F32, tag="qnat")
        nc.vector.memset(qnat, 0.0)
        nc.scalar.dma_start(out=qnat[:Q_SHARD, :], in_=query_d)

        for kt in range(4, KT):
            knat = knat_pool.tile([P, D_K], F32, tag="knat", name=f"knat_{kt}")
            dma_engs[kt % 3].dma_start(out=knat,
                                       in_=key_d[kt * P:(kt + 1) * P, :])
            knats.append(knat)

        # remaining weights (needed only after klT): one packed DMA
        w32 = singles.tile([P, 3, KO, D_ATT], F32, tag="w32")
        nc.gpsimd.dma_start(out=w32, in_=w3_d)
        w16 = singles.tile([P, 3, KO, D_ATT], F16, tag="w16")
        nc.vector.tensor_copy(out=w16, in_=w32)
        WLQ, WBQ, WBK = 0, 1, 2

        identity = singles.tile([P, P], F32)
        make_identity(nc, identity)
        id16 = singles.tile([P, P], F16, tag="id16")
        nc.vector.tensor_copy(out=id16, in_=identity)

        # 64 aligned stationaries: vsta[:, q, :] is [128, 64] with v_att in
        # column q, zeros elsewhere. Built with one strided broadcast copy
        # (positions q*64+q = q*65 in the flattened view). Aligned slabs --
        # a sliding-window variant with per-q 2-byte offsets ran ~5x slower
        # on HW. fp16 so the linear-path matmuls stream at 1 cycle/row.
        vsta = singles.tile([P, Q_SHARD, Q_SHARD], F16, tag="vsta")
        nc.vector.memset(vsta, 0.0)
        vsta_flat = vsta.rearrange("p a b -> p (a b)")
        nc.vector.tensor_copy(out=vsta_flat[:, 0:Q_SHARD * Q_SHARD:Q_SHARD + 1],
                              in_=vT.to_broadcast((P, Q_SHARD)))

        # ---- query transposes + projections (fp32: tiny and off the
        # critical path, so keep full precision on the q side) ----
        qT = singles.tile([P, KO, Q_SHARD], F32, tag="qT")
        pbq = wk_psum.tile([P, KO * P], F32, tag="wk", name="ptq")
        for c in range(KO):
            nc.tensor.transpose(pbq[:, c * P:(c + 1) * P],
                                qnat[:, c * P:(c + 1) * P], identity)
        nc.vector.tensor_copy(
            out=qT, in_=pbq.rearrange("p (c k) -> p c k", c=KO)[:, :, :Q_SHARD])

        qlT = singles.tile([P, Q_SHARD], F32, tag="qlT")
        pql = wk_psum.tile([P, P], F32, tag="pj")
        for c in range(KO):
            nc.tensor.matmul(pql[:, :Q_SHARD], w32[:, WLQ, c, :], qT[:, c, :],
                             start=(c == 0), stop=(c == KO - 1))
        nc.vector.tensor_copy(out=qlT, in_=pql[:, :Q_SHARD])

        qbT = singles.tile([P, Q_SHARD], F16, tag="qbT")
        pqb = wk_psum.tile([P, P], F32, tag="pj")
        for c in range(KO):
            nc.tensor.matmul(pqb[:, :Q_SHARD], w32[:, WBQ, c, :], qT[:, c, :],
                             start=(c == 0), stop=(c == KO - 1))
        # qb scaled by 1/sqrt(D_ATT) (bilinear normalizer), bias first
        nc.vector.tensor_scalar(out=qbT, in0=pqb[:, :Q_SHARD], scalar1=bbq,
                                scalar2=1.0 / math.sqrt(D_ATT),
                                op0=mybir.AluOpType.add,
                                op1=mybir.AluOpType.mult)

        # ---- key pipeline per tile: 4 fp32 PE transposes into one PSUM
        # bank, a single fp16-casting evac (alternating DVE/ACT -- ACT only
        # absorbs what fits in its otherwise-idle prologue window), fp16
        # klT projection, DVE bias-fold ----
        klT = singles.tile([P, NUM_K], F32, tag="klT")
        kbT = singles.tile([P, NUM_K], F16, tag="kbT")
        keyts = []
        for kt in range(KT):
            pb = wk_psum.tile([P, KO * P], F32, tag="wk", name=f"ptk_{kt}")
            for c in range(KO):
                nc.tensor.transpose(pb[:, c * P:(c + 1) * P],
                                    knats[kt][:, c * P:(c + 1) * P], identity)
            keyt = keyt_pool.tile([P, KO, P], F16, tag="keyt",
                                  name=f"keyt_{kt}")
            pbv = pb.rearrange("p (c k) -> p c k", c=KO)
            if kt % 2 == 0:
                nc.vector.tensor_copy(out=keyt, in_=pbv)
            else:
                nc.scalar.copy(out=keyt, in_=pbv)
            keyts.append(keyt)
            pkl = wk_psum.tile([P, P], F32, tag="pj", name=f"pkl_{kt}")
            for c in range(KO):
                nc.tensor.matmul(pkl, wlk16[:, c, :], keyt[:, c, :],
                                 start=(c == 0), stop=(c == KO - 1))
            nc.vector.tensor_scalar_add(out=klT[:, kt * P:(kt + 1) * P],
                                        in0=pkl, scalar1=bsum)

        # ---- kbT projection (runs in the ACT pipeline's shadow) ----
        for kt in range(KT):
            pkb = wk_psum.tile([P, P], F32, tag="wk", name=f"pkb_{kt}")
            for c in range(KO):
                nc.tensor.matmul(pkb, w16[:, WBK, c, :], keyts[kt][:, c, :],
                                 start=(c == 0), stop=(c == KO - 1))
            nc.vector.tensor_scalar_add(out=kbT[:, kt * P:(kt + 1) * P],
                                        in0=pkb, scalar1=bbk)

        # ---- bilinear baseline into the 4 output psum banks ----
        import os as _os
        _tanh_only = bool(int(_os.environ.get("BENCH_TANH_ONLY", "0")))
        po = [out_psum.tile([Q_SHARD, 512], F32, tag="po", name=f"po_{i}")
              for i in range(NB)]
        for i in range(NB):
            nc.tensor.matmul(po[i], qbT, kbT[:, i * 512:(i + 1) * 512],
                             start=True, stop=(_tanh_only and i > 0))

        # ---- linear (tanh) path, accumulated on top ----
        import os
        tanh_only = bool(int(os.environ.get("BENCH_TANH_ONLY", "0")))
        for q in range(Q_SHARD):
            tmp = tmp_pool.tile([P, NUM_K], F16, tag="tmp")
            nc.scalar.activation(tmp, klT, mybir.ActivationFunctionType.Tanh,
                                 bias=qlT[:, q:q + 1], scale=1.0)
            last = q == Q_SHARD - 1
            sta = vsta[:, q, :]
            if not tanh_only:
                for i in range(NB):
                    nc.tensor.matmul(po[i], sta, tmp[:, i * 512:(i + 1) * 512],
                                     start=False, stop=last)
            elif last:
                # keep tmp consumed so the loop still drains
                nc.tensor.matmul(po[0], sta, tmp[:, 0:512],
                                 start=False, stop=True)

        # ---- + b_att, evacuate, store ----
        out_sb = singles.tile([Q_SHARD, NUM_K], F32, tag="out_sb")
        if tanh_only:
            nc.vector.memset(out_sb, 0.0)
        for i in range(NB):
            if not tanh_only:
                nc.vector.tensor_scalar_add(
                    out=out_sb[:, i * 512:(i + 1) * 512],
                    in0=po[i], scalar1=batt[:Q_SHARD])
        nc.sync.dma_start(out=out_d, in_=out_sb)

    nc.compile()
    return nc


def _get_nc() -> bass.Bass:
    if "nc" not in _CACHED:
        _CACHED["nc"] = _build_bass()
    return _CACHED["nc"]


def make_in_maps(**inputs) -> list[dict[str, np.ndarray]]:
    f = lambda x: np.ascontiguousarray(np.asarray(x, dtype=np.float32))
    query = f(inputs["query"])
    key = f(inputs["key"])
    # pre-pack weights partition-major: [ko*128+p, d] -> [p, ko, d]
    pack = lambda w: np.ascontiguousarray(
        f(w).reshape(KO, P, D_ATT).transpose(1, 0, 2))
    wlk = pack(inputs["W_lk"])
    w3 = np.ascontiguousarray(np.stack(
        [pack(inputs["W_lq"]), pack(inputs["W_bq"]), pack(inputs["W_bk"])],
        axis=1))  # [128, 3, 4, 128]
    vec = np.zeros((6, D_ATT), np.float32)
    vec[0] = f(inputs["b_lq"]) + f(inputs["b_lk"])
    vec[1] = f(inputs["b_bq"])
    vec[2] = f(inputs["b_bk"])
    vec[3] = f(inputs["v_att"])
    vec[4] = np.float32(np.asarray(inputs["b_att"], np.float32).reshape(()))
    vec = np.ascontiguousarray(vec.T)  # [128, 6]
    shared = {"wlk": wlk, "w3": w3, "vec": vec}
    in_maps = []
    for c in range(N_CORES):
        b, qc = divmod(c, Q_CHUNKS)
        in_maps.append({
            "query_s": np.ascontiguousarray(query[b, qc * Q_SHARD:(qc + 1) * Q_SHARD, :]),
            "key_b": np.ascontiguousarray(key[b]),
            **shared,
        })
    return in_maps


def assemble(results: list[dict[str, np.ndarray]]) -> np.ndarray:
    out = np.empty((BSZ, NUM_Q, NUM_K), np.float32)
    for c in range(N_CORES):
        b, qc = divmod(c, Q_CHUNKS)
        out[b, qc * Q_SHARD:(qc + 1) * Q_SHARD, :] = results[c]["out"]
    return out


def kernel(**inputs) -> np.ndarray:
    nc = _get_nc()
    in_maps = make_in_maps(**inputs)
    res = run_bass_kernel_spmd(nc, in_maps, list(range(N_CORES)))
    return assemble(res.results)
